# revision 22
# baseline (speedup 1.0000x reference)
"""Causal GQA self-attention block (B=2, T=2048, C=2048, 16 q-heads / 4 kv-heads,
head_dim=128, RoPE + RMS-norm on q/k) for 8 Trainium2 NeuronCores.

Sharding: core = (batch b, kv-group g), b in {0,1}, g in {0..3}.
Each core computes its batch's projections for 4 q-heads + 1 kv head,
causal attention, and a partial output projection (Wo row-shard).
Host sums the 4 bf16 partials per batch.

v2 design notes (tuned against the TimelineSim cost model):
 - bf16 operands everywhere on the PE (same cycles/row as f32r at >=256
   moving, but half the DMA bytes, half the SBUF footprint, 1.0 c/r
   transposes, and 2x DVE throughput on 16-bit elementwise work).
 - x^T, weights, cos/sin tables fully resident in SBUF; input DMA is
   front-loaded in consumption order so the PE never waits mid-phase.
 - K's rms-norm scale is folded into the softmax exp() via a per-partition
   scale AP (rsk = SCALE/rms(k_row)), so K is never explicitly normalized.
 - causal mask applied multiplicatively on exp() output (bf16, 2x DVE)
   instead of additively on fp32 scores in PSUM.
 - softmax reciprocal broadcast across partitions on the Pool engine
   (partition_broadcast) instead of a PE ones-matmul.
 - PSUM->SBUF evacuations split between the Scalar (ACT) and Vector
   engines; RoPE multiplies on the Pool engine.
 - one shared 2-bank PSUM slot pool carries q+kv projections, score
   pairs, and out-projection chunks (PSUM is exactly 8 banks).
 - exp() is batched over pairs of full score blocks to amortize the ACT
   engine's SBUF access overhead (ACT is the regional bottleneck).
 - phase interleaving: attention for query chunk c2 is emitted after
   projection block 4*c2+4; out-projection chunks of c2 are spread among
   the following projection blocks, keeping every engine fed.
"""

import sys

for _p in ("/opt/trn_rl_repo", "/root/.axon_site/_ro/trn_rl_repo"):
    if _p not in sys.path:
        sys.path.insert(0, _p)

import numpy as np
import ml_dtypes

import concourse.bass as bass  # noqa: F401
import concourse.mybir as mybir
from concourse import bacc
from concourse.tile import TileContext
from concourse.bass_utils import run_bass_kernel_spmd

P = 128
T = 2048
C = 2048
KT = C // P          # 16 contraction tiles
TB = T // P          # 16 T1 blocks
NH = 4               # q heads per core
D = 128              # head dim
SCALE = 1.0 / np.sqrt(D)
EPS = float(np.finfo(np.float32).eps)

BF16 = mybir.dt.bfloat16
F32 = mybir.dt.float32
F32R = mybir.dt.float32r
AF = mybir.ActivationFunctionType
ALU = mybir.AluOpType

_NC_CACHE = None


def build_nc():
    nc = bacc.Bacc("TRN2", target_bir_lowering=False, debug=False)

    # Pin every activation to the one table set covering all functions this
    # kernel uses (Exp, Ln, Copy); the default first-match set selection
    # would thrash LoadActFuncSet between the exp and ln sets (~1.3us per
    # reload on the ACT engine). Indices are preserved so the emitted
    # act_func_set_id still refers to the real act_info.json table.
    import concourse.bacc as _bacc_mod
    _orig_tables = _bacc_mod.get_activation_tables

    def _pinned_tables(arch):
        t = _orig_tables(arch)
        keep = "natural_log_exp_and_others"
        assert keep in t
        return {k: (v if k == keep else type(v)()) for k, v in t.items()}

    _bacc_mod.get_activation_tables = _pinned_tables
    try:
        return _build_nc_inner(nc)
    finally:
        _bacc_mod.get_activation_tables = _orig_tables


def _build_nc_inner(nc):

    xt = nc.dram_tensor("xt", [C, T], BF16, kind="ExternalInput")
    wq = nc.dram_tensor("wq", [C, NH * D], BF16, kind="ExternalInput")
    wkv = nc.dram_tensor("wkv", [C, 2 * D], BF16, kind="ExternalInput")
    wo = nc.dram_tensor("wo", [NH * D, C], BF16, kind="ExternalInput")
    ca4 = nc.dram_tensor("ca4", [P, TB * NH * D], BF16, kind="ExternalInput")
    cb4 = nc.dram_tensor("cb4", [P, TB * NH * D], BF16, kind="ExternalInput")
    consts = nc.dram_tensor("consts", [P, 2 * P + 1], BF16, kind="ExternalInput")
    epsq = nc.dram_tensor("epsq", [P, 1], F32, kind="ExternalInput")
    y = nc.dram_tensor("y", [T, C], BF16, kind="ExternalOutput")

    xt_v = xt.ap().rearrange("(co ci) t -> ci co t", ci=P)
    wq_v = wq.ap().rearrange("(co ci) n -> ci co n", ci=P)
    wkv_v = wkv.ap().rearrange("(co ci) n -> ci co n", ci=P)
    wo_v = wo.ap().rearrange("(h d) n -> d h n", d=P)
    ca4_v = ca4.ap().rearrange("p (tb r d) -> p tb r d", tb=TB, r=NH)
    cb4_v = cb4.ap().rearrange("p (tb r d) -> p tb r d", tb=TB, r=NH)

    with TileContext(nc, pool_alloc_mode="queue") as tc:
        with tc.tile_pool(name="resident", bufs=1) as wpool:
            xt_sb = wpool.tile([P, KT, T], BF16)
            wq_sb = wpool.tile([P, KT, NH * D], BF16)
            wkv_sb = wpool.tile([P, KT, 2 * D], BF16)
            wo_sb = wpool.tile([P, NH, C], BF16)
            ca_sb = wpool.tile([P, TB, NH, D], BF16)
            cb_sb = wpool.tile([P, TB, NH, D], BF16)
            kt_sb = wpool.tile([P, T], BF16)
            v_sb = wpool.tile([P, TB, D], BF16)
            qt_sb = wpool.tile([P, NH, T], BF16)
            consts_sb = wpool.tile([P, 2 * P + 1], BF16)
            epsq_sb = wpool.tile([P, 1], F32)
            dmask_sb = consts_sb[:, 0:P]
            ident_sb = consts_sb[:, P:2 * P]
            onesc_sb = consts_sb[:, 2 * P:2 * P + 1]

            # ---- input DMA, in consumption order -------------------------
            # Each dma_start costs ~625ns on the shared HWDGE issue device,
            # so transfers are batched into few large DMAs (48 total) ordered
            # by first consumption.
            nc.sync.dma_start(out=wq_sb[:, 0, :], in_=wq_v[:, 0, :])
            nc.sync.dma_start(out=xt_sb[:, 0, 0:256], in_=xt_v[:, 0, 0:256])
            for kg in range(4):
                k0 = 1 if kg == 0 else 4 * kg
                nc.sync.dma_start(out=wq_sb[:, k0:4 * kg + 4, :],
                                  in_=wq_v[:, k0:4 * kg + 4, :])
                nc.sync.dma_start(out=xt_sb[:, k0:4 * kg + 4, 0:256],
                                  in_=xt_v[:, k0:4 * kg + 4, 0:256])
            nc.sync.dma_start(out=consts_sb[:], in_=consts.ap())
            nc.sync.dma_start(out=epsq_sb[:], in_=epsq.ap())
            nc.sync.dma_start(out=wkv_sb[:], in_=wkv_v[:])
            for cg in range(4):
                nc.sync.dma_start(out=xt_sb[:, 4 * cg:4 * cg + 4, 256:512],
                                  in_=xt_v[:, 4 * cg:4 * cg + 4, 256:512])
            nc.sync.dma_start(out=ca_sb[:, 0:4], in_=ca4_v[:, 0:4])
            nc.sync.dma_start(out=cb_sb[:, 0:4], in_=cb4_v[:, 0:4])
            nc.sync.dma_start(out=ca_sb[:, 4:8], in_=ca4_v[:, 4:8])
            nc.sync.dma_start(out=cb_sb[:, 4:8], in_=cb4_v[:, 4:8])
            for cg in range(4):
                nc.sync.dma_start(out=xt_sb[:, 4 * cg:4 * cg + 4, 512:1024],
                                  in_=xt_v[:, 4 * cg:4 * cg + 4, 512:1024])
            nc.sync.dma_start(out=ca_sb[:, 8:16], in_=ca4_v[:, 8:16])
            nc.sync.dma_start(out=cb_sb[:, 8:16], in_=cb4_v[:, 8:16])
            for cg in range(4):
                nc.sync.dma_start(out=xt_sb[:, 4 * cg:4 * cg + 4, 1024:2048],
                                  in_=xt_v[:, 4 * cg:4 * cg + 4, 1024:2048])
            nc.sync.dma_start(out=wo_sb[:, 0:2, :], in_=wo_v[:, 0:2, :])
            nc.sync.dma_start(out=wo_sb[:, 2:4, :], in_=wo_v[:, 2:4, :])

            with tc.tile_pool(name="p1sb", bufs=2) as p1sb, \
                 tc.tile_pool(name="p1small", bufs=2) as p1small, \
                 tc.tile_pool(name="pu", bufs=2, space="PSUM") as pu, \
                 tc.tile_pool(name="p1t", bufs=1, space="PSUM") as p1t, \
                 tc.tile_pool(name="p2pt", bufs=2) as p2pt, \
                 tc.tile_pool(name="p2small", bufs=2) as p2small, \
                 tc.tile_pool(name="p2rb", bufs=2) as p2rb, \
                 tc.tile_pool(name="potc", bufs=2) as potc, \
                 tc.tile_pool(name="p3ysb", bufs=2) as p3ysb, \
                 tc.tile_pool(name="p2o", bufs=2, space="PSUM") as p2o, \
                 tc.tile_pool(name="p2l", bufs=1, space="PSUM") as p2l:

                # One shared 2-bank rotating PSUM slot pool (same tag => same
                # buffers) carries: q+kv projection accumulators, score-block
                # pairs, and y out-projection chunk pairs. 2x2 + 1 + 2 + 1 = 8
                # PSUM banks total.
                def u2():
                    return pu.tile([P, 2, 512], F32, tag="u", name="u2")

                p1_state = {}

                def phase1_chains(tb):
                    """projections + rope + rms for t1 block tb (everything
                    except the PE transposes, which are deferred so the PE
                    queue never parks on this block's ACT/Pool/DVE chain)"""
                    uqkv = u2()
                    q_ps = uqkv[:, 0, :]
                    kv_ps = uqkv[:, 1, 0:2 * D]
                    for ki in range(KT):
                        nc.tensor.matmul(q_ps, xt_sb[:, ki, tb * P:(tb + 1) * P],
                                         wq_sb[:, ki, :],
                                         start=(ki == 0), stop=(ki == KT - 1))
                    for ki in range(KT):
                        nc.tensor.matmul(kv_ps, xt_sb[:, ki, tb * P:(tb + 1) * P],
                                         wkv_sb[:, ki, :],
                                         start=(ki == 0), stop=(ki == KT - 1))

                    # PSUM evacuation on ACT (casts to bf16)
                    q_sb = p1sb.tile([P, NH, D], BF16, tag="q_sb")
                    k_sb = p1sb.tile([P, D], BF16, tag="k_sb")
                    nc.scalar.copy(out=q_sb[:], in_=q_ps.rearrange(
                        "p (h d) -> p h d", h=NH))
                    nc.scalar.copy(out=k_sb[:], in_=kv_ps[:, 0:D])
                    nc.scalar.copy(out=v_sb[:, tb, :], in_=kv_ps[:, D:2 * D])

                    # rope: ca = [cos|sin], cb = [sin|cos] (per-block tiles,
                    # q-side replicated 4x along heads). Multiplies on Pool.
                    pa = p1sb.tile([P, NH, D], BF16, tag="pa")
                    pb = p1sb.tile([P, NH, D], BF16, tag="pb")
                    pka = p1sb.tile([P, D], BF16, tag="pka")
                    pkb = p1sb.tile([P, D], BF16, tag="pkb")
                    nc.vector.tensor_mul(pa[:], q_sb[:], ca_sb[:, tb])
                    nc.vector.tensor_mul(pb[:], q_sb[:], cb_sb[:, tb])
                    nc.gpsimd.tensor_mul(pka[:], k_sb[:], ca_sb[:, tb, 0, :])
                    nc.gpsimd.tensor_mul(pkb[:], k_sb[:], cb_sb[:, tb, 0, :])

                    rq = p1sb.tile([P, NH, D], BF16, tag="rq")
                    rk = p1sb.tile([P, D], BF16, tag="rk")
                    nc.vector.tensor_add(rq[:, :, 0:64], pa[:, :, 0:64], pa[:, :, 64:128])
                    nc.vector.tensor_sub(rq[:, :, 64:128], pb[:, :, 64:128], pb[:, :, 0:64])
                    nc.vector.tensor_add(rk[:, 0:64], pka[:, 0:64], pka[:, 64:128])
                    nc.vector.tensor_sub(rk[:, 64:128], pkb[:, 64:128], pkb[:, 0:64])

                    # rms-norm factors (sum of squares via stt accum).
                    # k's factor folds in SCALE and is consumed by exp() later.
                    sqs = p1sb.tile([P, NH, D], BF16, tag="sqs", bufs=1)
                    sqk = p1sb.tile([P, D], BF16, tag="sqk", bufs=1)
                    ss = p1small.tile([P, 8], F32, tag="ss")
                    for h in range(NH):
                        nc.vector.scalar_tensor_tensor(
                            out=sqs[:, h, :], in0=rq[:, h, :], scalar=1.0,
                            in1=rq[:, h, :], op0=ALU.mult, op1=ALU.mult,
                            accum_out=ss[:, h:h + 1])
                    nc.vector.scalar_tensor_tensor(
                        out=sqk[:], in0=rk[:], scalar=1.0,
                        in1=rk[:], op0=ALU.mult, op1=ALU.mult,
                        accum_out=ss[:, NH:NH + 1])
                    # rs = (ms+eps)^-1/2 via exp(-0.5*ln(.)): ln/exp/copy
                    # share one ACT function set, so no LoadActFuncSet thrash
                    sq = p1small.tile([P, 8], F32, tag="sq")
                    nc.scalar.activation(sq[:, 0:NH + 1], ss[:, 0:NH + 1], AF.Ln,
                                         bias=epsq_sb[:], scale=1.0 / D)
                    rs = p1small.tile([P, 8], F32, tag="rs")
                    nc.scalar.activation(rs[:, 0:NH + 1], sq[:, 0:NH + 1], AF.Exp,
                                         scale=-0.5)

                    qn = p1sb.tile([P, NH, D], BF16, tag="qn", bufs=3)
                    kn = p1sb.tile([P, D], BF16, tag="kn", bufs=3)
                    for h in range(NH):
                        nc.vector.tensor_scalar_mul(qn[:, h, :], rq[:, h, :],
                                                    rs[:, h:h + 1])
                    nc.vector.tensor_scalar_mul(kn[:], rk[:], rs[:, NH:NH + 1])

                    p1_state[tb] = (qn, kn)

                def phase1_tail(tb):
                    """transposes -> [D, T1] layout (bf16, through PSUM) and
                    the qt/kt SBUF copies for t1 block tb"""
                    qn, kn = p1_state.pop(tb)
                    t_ps = p1t.tile([P, 5, D], BF16)
                    for h in range(NH):
                        nc.tensor.transpose(t_ps[:, h, :], qn[:, h, :], ident_sb[:])
                    nc.tensor.transpose(t_ps[:, NH, :], kn[:], ident_sb[:])
                    # k first (attention m-loops hit it first); per-head q
                    # copies so the first head of a chunk unblocks early
                    nc.vector.tensor_copy(out=kt_sb[:, tb * P:(tb + 1) * P],
                                          in_=t_ps[:, NH, :])
                    for h in range(NH):
                        nc.vector.tensor_copy(
                            out=qt_sb[:, h, tb * P:(tb + 1) * P],
                            in_=t_ps[:, h, :])

                ot_tiles = {}

                def attention(c2, fillers=()):
                    """S^T-layout causal attention for query chunk c2 (512 wide),
                    all 4 heads. Score-block pairs share one 2-bank PSUM slot;
                    exp() is batched per pair; the score matmuls of the next
                    pair are emitted before this pair's PV/l matmuls so the
                    exp() latency is always hidden (lookahead-1, also across
                    head boundaries). `fillers` (next chunk's projection
                    blocks, previous chunk's out-projections) are woven
                    between pairs: their long PE chains cover the ACT/DVE
                    latency of the attention pipeline."""
                    ot_sb = potc.tile([P, NH, 512], BF16, tag="ot")
                    ot_tiles[c2] = ot_sb
                    npair = (4 * c2 + 4) // 2
                    m_last = 4 * c2 + 3

                    def emit_scores(h, g):
                        s2 = u2()
                        pt2 = p2pt.tile([P, 2, 512], BF16, name="pt2")
                        for i, m in enumerate((2 * g, 2 * g + 1)):
                            dg = m - 4 * c2
                            n0 = 128 * dg if dg > 0 else 0
                            nc.tensor.matmul(
                                s2[:, i, n0:512],
                                kt_sb[:, m * P:(m + 1) * P],
                                qt_sb[:, h, c2 * 512 + n0:(c2 + 1) * 512],
                                start=True, stop=True)
                        # one exp per pair when the wasted columns are cheaper
                        # than a second ACT instruction init
                        if 2 * g + 1 <= 4 * c2 + 1:
                            nc.scalar.activation(pt2[:], s2[:], AF.Exp,
                                                 scale=float(SCALE))
                        else:
                            for i, m in enumerate((2 * g, 2 * g + 1)):
                                n0 = 128 * (m - 4 * c2)
                                nc.scalar.activation(
                                    pt2[:, i, n0:512], s2[:, i, n0:512], AF.Exp,
                                    scale=float(SCALE))
                        return pt2

                    def emit_pv(h, g, pt2, o_ps, l_ps):
                        for i, m in enumerate((2 * g, 2 * g + 1)):
                            dg = m - 4 * c2
                            n0 = 128 * dg if dg > 0 else 0
                            if dg >= 0:
                                # causal mask on the diagonal 128x128 block
                                nc.vector.tensor_mul(pt2[:, i, n0:n0 + P],
                                                     pt2[:, i, n0:n0 + P],
                                                     dmask_sb)
                            nc.tensor.matmul(o_ps[:, n0:512], v_sb[:, m, :],
                                             pt2[:, i, n0:512],
                                             start=(m == 0), stop=(m == m_last),
                                             skip_group_check=True)
                            nc.tensor.matmul(l_ps[:, n0:512], onesc_sb,
                                             pt2[:, i, n0:512],
                                             start=(m == 0), stop=(m == m_last),
                                             skip_group_check=True)

                    def normalize(h, o_ps, l_ps):
                        # 1/l broadcast across partitions on Pool
                        l_sb = p2small.tile([1, 512], F32, tag="l_sb")
                        nc.vector.tensor_copy(out=l_sb[:], in_=l_ps[:])
                        rl = p2small.tile([1, 512], F32, tag="rl")
                        nc.vector.reciprocal(rl[:], l_sb[:])
                        rb_sb = p2rb.tile([P, 512], F32)
                        nc.gpsimd.partition_broadcast(rb_sb[:], rl[:])
                        nc.vector.tensor_mul(ot_sb[:, h, :], o_ps[:], rb_sb[:])

                    fillers = list(fillers)
                    if fillers:
                        fillers.pop(0)()  # pre-slot: covers last-block qt latency
                    nf = len(fillers)
                    total = NH * npair
                    pending = None  # (h, g, pt2, o_ps, l_ps)
                    fi = 0
                    j = 0
                    for h in range(NH):
                        new_o = p2o.tile([P, 512], F32, name="o_ps")
                        new_l = p2l.tile([1, 512], F32, name="l_ps")
                        for g in range(npair):
                            pt2 = emit_scores(h, g)
                            j += 1
                            while fi < nf and j * nf >= (fi + 1) * total:
                                fillers[fi]()
                                fi += 1
                            if pending is not None:
                                emit_pv(*pending)
                                if pending[1] == npair - 1:
                                    normalize(pending[0], pending[3], pending[4])
                            pending = (h, g, pt2, new_o, new_l)
                    emit_pv(*pending)
                    normalize(pending[0], pending[3], pending[4])
                    while fi < nf:
                        fillers[fi]()
                        fi += 1

                def yproj(c2, tq, split_dma=False):
                    """out-projection for t1 block c2*4+tq; 4 PSUM chunks are
                    copied (ACT/DVE alternating) into one [P, 2048] row tile
                    and stored with a single DMA (split per chunk for the
                    final block so the end-of-kernel drain is short)."""
                    ot_sb = ot_tiles[c2]
                    tt = c2 * 4 + tq
                    y_sb = p3ysb.tile([P, 4, 512], BF16, name="y_sb")
                    for cp in range(2):
                        yu = u2()
                        for i in range(2):
                            co = 2 * cp + i
                            y_ps = yu[:, i, :]
                            for h in range(NH):
                                nc.tensor.matmul(
                                    y_ps,
                                    ot_sb[:, h, tq * P:(tq + 1) * P],
                                    wo_sb[:, h, co * 512:(co + 1) * 512],
                                    start=(h == 0), stop=(h == NH - 1))
                            if co % 2 == 0:
                                nc.scalar.copy(out=y_sb[:, co, :], in_=y_ps)
                            else:
                                nc.vector.tensor_copy(out=y_sb[:, co, :], in_=y_ps)
                            if split_dma:
                                nc.sync.dma_start(
                                    out=y.ap()[tt * P:(tt + 1) * P,
                                               co * 512:(co + 1) * 512],
                                    in_=y_sb[:, co, :])
                    if not split_dma:
                        nc.sync.dma_start(
                            out=y.ap()[tt * P:(tt + 1) * P, :], in_=y_sb[:])

                # ---- interleaved schedule ----
                # ATT(c2) consumes blocks <= 4*c2+3 and weaves in, as fillers,
                # the next chunk's projection blocks and the previous chunk's
                # out-projections.
                def chn(tb):
                    return lambda: phase1_chains(tb)

                def tl(tb):
                    return lambda: phase1_tail(tb)

                def ypr(c2, tq):
                    return lambda: yproj(c2, tq)

                phase1_chains(0)
                phase1_chains(1)
                phase1_chains(2)
                phase1_tail(0)
                phase1_chains(3)
                phase1_tail(1)
                phase1_tail(2)
                phase1_tail(3)
                attention(0, [chn(4), chn(5), chn(6), tl(4), chn(7), tl(5),
                              tl(6), tl(7)])
                attention(1, [chn(8), ypr(0, 0), chn(9), ypr(0, 1), tl(8),
                              chn(10), ypr(0, 2), tl(9), chn(11), ypr(0, 3),
                              tl(10), tl(11)])
                attention(2, [chn(12), ypr(1, 0), chn(13), ypr(1, 1), tl(12),
                              chn(14), ypr(1, 2), tl(13), chn(15), ypr(1, 3),
                              tl(14), tl(15)])
                attention(3, [ypr(2, 0), ypr(2, 1), ypr(2, 2), ypr(2, 3)])
                for tq in range(4):
                    yproj(3, tq, split_dma=(tq == 3))

    nc.compile()
    return nc


def make_in_maps(x, cos, sin, Wq, Wk, Wv, Wo):
    """Build per-core input maps (host-side numpy preprocessing, shared
    arrays across cores wherever possible)."""
    bf16 = ml_dtypes.bfloat16
    # x^T per batch (shared by the 4 kv-group cores of that batch)
    xts = [np.ascontiguousarray(x[b].T).astype(bf16) for b in range(2)]

    cos2 = np.ascontiguousarray(cos.reshape(T, 64)).astype(np.float32)
    sin2 = np.ascontiguousarray(sin.reshape(T, 64)).astype(np.float32)
    csa = np.concatenate([cos2, sin2], axis=1)          # [T, 128]
    csb = np.concatenate([sin2, cos2], axis=1)          # [T, 128]
    # replicate to [ti, tb, rep, d] resident layout
    ca4 = np.ascontiguousarray(np.broadcast_to(
        csa.reshape(TB, P, 1, P).transpose(1, 0, 2, 3), (P, TB, NH, P))
    ).reshape(P, TB * NH * P).astype(bf16)
    cb4 = np.ascontiguousarray(np.broadcast_to(
        csb.reshape(TB, P, 1, P).transpose(1, 0, 2, 3), (P, TB, NH, P))
    ).reshape(P, TB * NH * P).astype(bf16)

    pp, yy = np.meshgrid(np.arange(P), np.arange(P), indexing="ij")
    dmask = np.where(yy >= pp, 1.0, 0.0).astype(bf16)   # keep t1 >= t2
    ident = np.eye(P, dtype=np.float32).astype(bf16)
    onesc = np.ones((P, 1), dtype=bf16)
    consts = np.ascontiguousarray(
        np.concatenate([dmask, ident, onesc], axis=1)).astype(bf16)
    epsq = np.full((P, 1), EPS, dtype=np.float32)

    wqs, wkvs, wos = [], [], []
    for g in range(4):
        wqs.append(np.ascontiguousarray(Wq[:, 512 * g:512 * (g + 1)]).astype(bf16))
        wkvs.append(np.ascontiguousarray(
            np.concatenate([Wk[:, 128 * g:128 * (g + 1)],
                            Wv[:, 128 * g:128 * (g + 1)]], axis=1)).astype(bf16))
        wos.append(np.ascontiguousarray(Wo[512 * g:512 * (g + 1), :]).astype(bf16))

    in_maps = []
    for core in range(8):
        b, g = divmod(core, 4)
        in_maps.append({
            "xt": xts[b],
            "wq": wqs[g],
            "wkv": wkvs[g],
            "wo": wos[g],
            "ca4": ca4,
            "cb4": cb4,
            "consts": consts,
            "epsq": epsq,
        })
    return in_maps


def kernel(x, cos, sin, Wq, Wk, Wv, Wo):
    global _NC_CACHE
    x = np.asarray(x, dtype=np.float32)
    cos = np.asarray(cos, dtype=np.float32)
    sin = np.asarray(sin, dtype=np.float32)
    Wq = np.asarray(Wq, dtype=np.float32)
    Wk = np.asarray(Wk, dtype=np.float32)
    Wv = np.asarray(Wv, dtype=np.float32)
    Wo = np.asarray(Wo, dtype=np.float32)

    if _NC_CACHE is None:
        _NC_CACHE = build_nc()
    nc = _NC_CACHE

    in_maps = make_in_maps(x, cos, sin, Wq, Wk, Wv, Wo)
    res = run_bass_kernel_spmd(nc, in_maps, core_ids=list(range(8)))

    out = np.zeros((2, T, C), dtype=np.float32)
    for core in range(8):
        b = core // 4
        out[b] += res.results[core]["y"].astype(np.float32)
    return out
